# revision 24
# baseline (speedup 1.0000x reference)
"""BiTreeLSTM forward pass on 8 TRN2 NeuronCores.

Strategy (8-way tensor parallel on the hidden/gate dimension):
  - Core k owns hidden dims [128k, 128k+128) -> a 768-row slice of the 6144
    gate rows (6 gate types x 128 dims) plus matching slices of c/h/px.
  - Input projections x_gates = feat @ Wx.T and px = feat @ Wpx.T are computed
    on-device as a bf16 GEMM split across cores by output columns; biases are
    folded into the PSUM->SBUF copy (vector add against a per-gate bias tile).
  - The leaf-to-root recurrence is batched by tree level.  Levels are split
    into "chunks" (kid-aligned halves for the two big levels) and each chunk
    runs: ident-matmul x_gates injection (pre-issued so it executes during the
    previous AllGather), 16 K-chunk matmuls accumulating Wlr.T @ [h_l | h_r]
    into PSUM (gate-major [128, B]), ScalarE sigmoid/tanh, VectorE c/h chain
    writing h directly in bf16, then an 8-core AllGather replicates the new h
    slices into every core's h table.  Stage DMA + collective trigger +
    scatter all issue from GpSimd to avoid engine wake-up gaps.
  - Region-0 GEMM subtiles are deferred between tail levels to keep PE busy
    during collective waits.
  - Gate chunk order is [i, o, fl, fr, r, u] so the five sigmoid gates form
    one contiguous activation span.
  - All transposes / bf16 casts are host-side numpy.  The level schedule is
    compiled from the actual child-index inputs.
"""

import sys

import numpy as np

_REPO = "/opt/trn_rl_repo"
if _REPO not in sys.path:
    sys.path.insert(0, _REPO)

import ml_dtypes  # noqa: E402

import concourse.bass as bass  # noqa: E402,F401
import concourse.mybir as mybir  # noqa: E402
import concourse.tile as tile  # noqa: E402
from concourse import bacc  # noqa: E402
from concourse.bass_utils import run_bass_kernel_spmd  # noqa: E402

NCORES = 8
H = 1024
HS = H // NCORES          # 128 hidden dims per core
S6 = 6 * HS               # 768 gate rows per core
HC = H // 128             # 8 h-table chunks
KC = 2 * H // 128         # 16 recurrence K chunks
SEG = 512                 # max nodes per psum chunk (recurrence levels)
FTW = 256                 # feat tile width (node columns per GEMM tile)

AF = mybir.ActivationFunctionType
ALU = mybir.AluOpType
BF16 = mybir.dt.bfloat16
F32 = mybir.dt.float32
BF16_NP = ml_dtypes.bfloat16

# per-core gate chunk order: i, o, f_l, f_r, r, u  (u = tanh is last so the
# five sigmoid chunks are contiguous); original stack order is i,o,fl,fr,u,r
GATE_PERM = [0, 1, 2, 3, 5, 4]
G_I, G_O, G_FL, G_FR, G_R, G_U = range(6)


# ---------------------------------------------------------------- schedule --

def _runs(vals, limit, region=None):
    """Decompose an int list into (start, step, count, pos) arithmetic runs.

    A run is kept only if [start, start+step*count) stays within `limit` and
    step >= 1; otherwise singletons.  With `region`, runs additionally never
    cross a multiple-of-region boundary (needed for region-tiled tables).
    """
    out = []
    i, m = 0, len(vals)
    while i < m:
        j = i + 1
        if j < m:
            step = vals[j] - vals[i]
            while j + 1 < m and vals[j + 1] - vals[j] == step:
                if region is not None and vals[j + 1] // region != vals[i] // region:
                    break
                j += 1
            if region is not None and vals[j] // region != vals[i] // region:
                while j > i and vals[j] // region != vals[i] // region:
                    j -= 1
            cnt = j - i + 1
            lim = limit
            if region is not None:
                lim = min(lim, (vals[i] // region + 1) * region)
            if cnt > 1 and step >= 1 and vals[i] + step * cnt <= lim:
                out.append((vals[i], step, cnt, i))
                i = j + 1
                continue
        out.append((vals[i], 1, 1, i))
        i += 1
    return out


def _mk_chunk(nodes, l, r, n, leaf):
    pad = n + 4
    ch = dict(bs=len(nodes), nodes=nodes,
              node_runs=_runs(nodes, n, region=FTW))
    if not leaf:
        ch["l_runs"] = _runs([int(l[i]) for i in nodes], pad)
        ch["r_runs"] = _runs([int(r[i]) for i in nodes], pad)
    return ch


def _schedule(left, right, n):
    """Levelize the tree exactly matching the reference scan semantics, then
    split each level into AllGather chunks.

    Reference processes i = n-1 .. 0; h_all[l] reads the computed value iff
    l > i, else the initial zero.  Remapping l<=i (or out of range) to the
    zero sentinel `n` makes all deps point to higher indices, so grouping by
    longest-path level gives a valid batched schedule.

    Levels 0 and 1 are split into two chunks with the split point chosen so
    chunk A of level 1 only reads h values covered by chunk A of level 0;
    each chunk gets its own AllGather, pipelining the collectives against the
    next chunk's compute.  Deeper levels are single chunks (<= SEG nodes).
    """
    idx = np.arange(n)
    l = np.asarray(left).astype(np.int64)
    r = np.asarray(right).astype(np.int64)
    l = np.where((l > idx) & (l >= 0) & (l <= n), l, n)
    r = np.where((r > idx) & (r >= 0) & (r <= n), r, n)
    lev = np.empty(n + 1, np.int64)
    lev[n] = -1
    for i in range(n - 1, -1, -1):
        lev[i] = 1 + max(lev[l[i]], lev[r[i]])
    nlev = int(lev[:n].max()) + 1
    node_lists = [np.where(lev[:n] == v)[0].tolist() for v in range(nlev)]

    levels = []
    for v, nodes in enumerate(node_lists):
        leaf = v == 0
        parts = [nodes]
        if v == 1 and len(nodes) >= 256 and len(node_lists[0]) >= 256:
            h1 = (len(nodes) + 1) // 2 - 1
            parts = [nodes[:h1], nodes[h1:]]
        elif v == 0 and len(nodes) >= 256 and len(node_lists) > 1 \
                and len(node_lists[1]) >= 256:
            # align the leaf split to cover the kids of level-1 chunk A
            l1 = node_lists[1]
            h1 = (len(l1) + 1) // 2 - 1
            kids = set()
            for i in l1[:h1]:
                kids.add(int(l[i])); kids.add(int(r[i]))
            pos = {nd: p for p, nd in enumerate(nodes)}
            kp = [pos[k] for k in kids if k in pos]
            if kp and max(kp) + 1 <= 0.75 * len(nodes):
                p = max(kp) + 1
                parts = [nodes[:p], nodes[p:]]
        elif not leaf and len(nodes) > SEG:
            parts = [nodes[s:s + SEG] for s in range(0, len(nodes), SEG)]
        chunks = [_mk_chunk(p, l, r, n, leaf) for p in parts if p]
        levels.append(dict(B=len(nodes), nodes=nodes, chunks=chunks))

    # Resolve kid runs to (src_level, src_chunk, rel_start, step, cnt, pos):
    # each chunk's h lives in its own SBUF tile laid out [128, HC*(bs+1)]
    # (chunk-major, one zero column at rel==bs for the sentinel), so matmul
    # rhs reads index into the producing chunk's tile directly.  Runs are
    # split at chunk boundaries.  Chunks must be dense integer ranges.
    spans = []  # (start, end_excl, li, ci, bs) per chunk, dense only
    for li, L in enumerate(levels):
        for ci, ch in enumerate(L["chunks"]):
            nd = ch["nodes"]
            if nd == list(range(nd[0], nd[0] + len(nd))):
                spans.append((nd[0], nd[0] + len(nd), li, ci, len(nd)))

    def locate(k, allow_sentinel):
        for (s, e, li, ci, bs) in spans:
            if s <= k < e:
                return (li, ci, bs, k - s)
            if allow_sentinel and k == n and e == n:
                return (li, ci, bs, bs)  # zero column
        raise ValueError(f"cannot resolve child index {k}")

    def resolve(runs, v):
        out = []
        for (a, st, cnt, pos) in runs:
            i = 0
            while i < cnt:
                li, ci, bs, rel = locate(a + st * i, a + st * i == n)
                if li >= v:
                    raise ValueError("child not in earlier level")
                # extend while the run stays inside this chunk's tile
                # (its trailing zero column at rel == bs serves k == n)
                j = i + 1
                while j < cnt:
                    r2 = rel + st * (j - i)
                    k2 = a + st * j
                    if r2 > bs or (r2 == bs and k2 != n) or \
                            (k2 != n and locate(k2, False)[:2] != (li, ci)):
                        break
                    j += 1
                out.append((li, ci, rel, st, j - i, pos + i))
                i = j
        return out

    for v, L in enumerate(levels):
        if v == 0:
            continue
        for ch in L["chunks"]:
            ch["l_src"] = resolve(ch["l_runs"], v)
            ch["r_src"] = resolve(ch["r_runs"], v)
    return levels


def _feat_tiles(levels, n):
    """GEMM subtiles (c0, w, key) ordered by first use (level, chunk).

    Regions whose columns are only needed by deep levels (>= 3) are split
    into >=32-wide subtiles keyed per level so their matmuls can be deferred
    into tail-level collective gaps.
    """
    NEED_BIG = (1 << 30, 0)
    need = [NEED_BIG] * n
    for v, L in enumerate(levels):
        for ci, ch in enumerate(L["chunks"]):
            for nd in ch["nodes"]:
                if (v, ci) < need[nd]:
                    need[nd] = (v, ci)
    tiles = []
    for c0 in range(0, n, FTW):
        w = min(FTW, n - c0)
        needs = need[c0:c0 + w]
        minlev = min(needs)
        if minlev[0] < 3:
            tiles.append((c0, w, minlev))
            continue
        # split by need-level, merging until >= 2 wide
        s = 0
        while s < w:
            e = s + 1
            while e < w and (needs[e][0] == needs[s][0] or e - s < 2):
                e += 1
            tiles.append((c0 + s, e - s, min(needs[s:e])))
            s = e
    tiles.sort(key=lambda t: (t[2], t[0]))
    return tiles


# ----------------------------------------------------------------- builder --

def _c3(ap2, a, cnt, step=1):
    """3D column view [P, cnt, 1] of ap2[:, a : a+step*cnt : step]."""
    if cnt == 1 or step == 1:
        return ap2[:, a:a + cnt].rearrange("p (k s) -> p k s", s=1)
    return ap2[:, a:a + step * cnt].rearrange("p (k s) -> p k s", s=step)[:, :, 0:1]


def build(nc, levels, feat_tiles, n, f):
    fc = f // 128
    pad = n + 4
    nlev = len(levels)
    gw = max(ch["bs"] for L in levels for ch in L["chunks"])
    hwA = max((ch["bs"] for L in levels[:2] for ch in L["chunks"]),
              default=1) + 1
    hwB = max((ch["bs"] for L in levels[2:] for ch in L["chunks"]),
              default=1) + 1
    nreg = (n + FTW - 1) // FTW
    # regions containing deferred subtiles get a persistent ft tile
    defer_regs = sorted({c0 // FTW for (c0, w, key) in feat_tiles
                         if key[0] >= 3})

    featT = nc.dram_tensor("featT", [f, n], BF16, kind="ExternalInput")
    wlrT = nc.dram_tensor("wlrT", [2 * H, S6], BF16, kind="ExternalInput")
    wxpxT = nc.dram_tensor("wxpxT", [f, S6 + HS], BF16, kind="ExternalInput")
    biasT = nc.dram_tensor("biasT", [128, 7], F32, kind="ExternalInput")
    ident = nc.dram_tensor("ident", [128, 128], BF16, kind="ExternalInput")
    out = nc.dram_tensor("out", [HS, n], BF16, kind="ExternalOutput")

    with tile.TileContext(nc) as tc:
        with (
            tc.tile_pool(name="wp", bufs=1) as wp,
            tc.tile_pool(name="tp", bufs=1) as tp,
            tc.tile_pool(name="fp", bufs=2) as fp,
            tc.tile_pool(name="ep", bufs=2) as ep,
            tc.tile_pool(name="sp", bufs=2) as sp,
            tc.tile_pool(name="hp", bufs=4) as hp,
            tc.tile_pool(name="hq", bufs=3) as hq,
            tc.tile_pool(name="pg", bufs=2, space="PSUM") as pgp,
            tc.tile_pool(name="pr", bufs=4, space="PSUM") as prp,
            tc.tile_pool(name="dp", bufs=2, space="DRAM") as dp,
        ):
            # ---- persistent SBUF ----
            wx_sb = wp.tile([128, fc * (S6 + HS)], BF16, name="wx_sb")
            wlr_sb = wp.tile([128, KC * S6], BF16, name="wlr_sb")
            ident_sb = wp.tile([128, 128], BF16, name="ident_sb")
            bias_sb = wp.tile([128, 7], F32, name="bias_sb")

            cT = tp.tile([HS, pad], F32, name="cT")
            xgr = [tp.tile([HS, 6 * FTW], BF16, name=f"xgr{i}")
                   for i in range(nreg)]
            pxr = [tp.tile([HS, FTW], F32, name=f"pxr{i}")
                   for i in range(nreg)]
            ft0 = {reg: tp.tile([128, fc * FTW], BF16, name=f"ft0_{reg}")
                   for reg in defer_regs}

            wlr_v = wlr_sb.rearrange("p (c x) -> p c x", c=KC)
            wx_v = wx_sb.rearrange("p (c x) -> p c x", c=fc)
            # per-chunk AllGathered h tiles: layout [p, hch*(bs+1) + b],
            # with a zero column at b == bs serving the sentinel index n.
            # The gathered-output DMA is issued lazily (scalar queue) right
            # before the first consumer's matmuls, so it never blocks other
            # chunks' activations behind it in the in-order queue.
            tmps = {}
            tmp_loaded = set()

            def load_tmp(key):
                if key in tmp_loaded:
                    return
                tmp_loaded.add(key)
                ttile, tbs, ago = tmps[key]
                t_v = ttile[:, 0:HC * (tbs + 1)].rearrange(
                    "p (c b) -> p c b", c=HC)
                nc.scalar.dma_start(
                    out=t_v[:, :, 0:tbs],
                    in_=ago.rearrange("(c p) b -> p c b", p=HS))

            # ---- loads (wx first so the GEMM can start asap) ----
            wx_d = wxpxT.rearrange("(c p) x -> p c x", p=128)
            nc.sync.dma_start(out=wx_v[:, :fc // 2, :], in_=wx_d[:, :fc // 2, :])
            nc.sync.dma_start(out=wx_v[:, fc // 2:, :], in_=wx_d[:, fc // 2:, :])
            nc.sync.dma_start(out=ident_sb[:, :], in_=ident[:, :])
            nc.sync.dma_start(out=bias_sb[:, :], in_=biasT[:, :])
            featT_v = featT.rearrange("(c p) x -> p c x", p=128)
            for reg in defer_regs:
                c0 = reg * FTW
                w = min(FTW, n - c0)
                nc.sync.dma_start(
                    out=ft0[reg].rearrange("p (c x) -> p c x", c=fc)[:, :, :w],
                    in_=featT_v[:, :, c0:c0 + w])
            nc.sync.dma_start(
                out=wlr_v[:, :, :],
                in_=wlrT.rearrange("(c p) x -> p c x", p=128))
            nc.vector.memset(cT[:, n:n + 1], 0.0)

            # dummy AllGathers to warm up the collective DMA rings while the
            # initial weight loads / first GEMM tiles run; the first
            # collectives otherwise pay ~50us of cold-start.
            wsrc = sp.tile([HS, 512], BF16, tag="wsrc", name="wsrc")
            nc.gpsimd.memset(wsrc[:, :], 0.0)
            for wi, wn in enumerate((512, 64)):
                wagi = dp.tile([HS, wn], BF16, tag="agin", name=f"wagi{wi}")
                wago = dp.tile([H, wn], BF16, tag="agout", name=f"wago{wi}",
                               addr_space="Shared")
                nc.gpsimd.dma_start(out=wagi[:, :], in_=wsrc[:, :wn])
                nc.gpsimd.collective_compute(
                    "AllGather", ALU.bypass,
                    replica_groups=[list(range(NCORES))],
                    ins=[wagi.opt()], outs=[wago.opt()])


            # ---- GEMM for one feat subtile (node cols c0..c0+w) ----
            def emit_gemm_tile(c0, w):
                reg = c0 // FTW
                loc = c0 % FTW
                if reg in ft0:
                    ft_v = ft0[reg].rearrange("p (c x) -> p c x", c=fc)
                    fl0 = loc
                else:
                    ftt = fp.tile([128, fc * FTW], BF16, tag="ft",
                                  name=f"ft_{c0}")
                    ft_v = ftt.rearrange("p (c x) -> p c x", c=fc)
                    nc.sync.dma_start(
                        out=ft_v[:, :, :w],
                        in_=featT_v[:, :, c0:c0 + w])
                    fl0 = 0
                for m in range(7):
                    ps = pgp.tile([128, FTW], F32, tag="pg", name=f"pg_{c0}_{m}")
                    for c in range(fc):
                        nc.tensor.matmul(
                            ps[:, :w],
                            lhsT=wx_v[:, c, m * 128:(m + 1) * 128],
                            rhs=ft_v[:, c, fl0:fl0 + w],
                            start=(c == 0), stop=(c == fc - 1))
                    if m < 6:
                        dst = xgr[reg][:, m * FTW + loc:m * FTW + loc + w]
                    else:
                        dst = pxr[reg][:, loc:loc + w]
                    nc.scalar.activation(dst, ps[:, :w], AF.Identity,
                                         bias=bias_sb[:, m:m + 1])

            # ---- one recurrence chunk (one AllGather unit) ----
            def emit_chunk(li, ci, ch):
                bs = ch["bs"]
                node_runs = ch["node_runs"]
                g = ep.tile([HS, 6 * gw], F32, tag="g", name=f"g_{li}_{ci}")

                if li == 0:
                    # leaves: gates come straight from (pre-biased) x_gates.
                    for (a, st, cnt, pos) in node_runs:
                        reg, loc = a // FTW, a % FTW
                        if st == 1:
                            # 5 sigmoid gates in one ACT via 3D views
                            xv = xgr[reg].rearrange("p (m x) -> p m x", m=6)
                            gsig = g[:, pos:pos + 5 * bs].rearrange(
                                "p (m x) -> p m x", m=5)[:, :, 0:cnt]
                            nc.scalar.activation(
                                gsig, xv[:, 0:5, loc:loc + cnt], AF.Sigmoid)
                            nc.scalar.activation(
                                g[:, G_U * bs + pos:G_U * bs + pos + cnt],
                                xv[:, 5, loc:loc + cnt], AF.Tanh)
                        else:
                            for m in range(6):
                                nc.scalar.activation(
                                    _c3(g, m * bs + pos, cnt),
                                    _c3(xgr[reg], m * FTW + loc, cnt, st),
                                    AF.Tanh if m == G_U else AF.Sigmoid)
                else:
                    cpp = min(6, max(1, SEG // bs))
                    nps = -(-6 // cpp)
                    psl = [prp.tile([128, SEG], F32, tag="pr",
                                    name=f"pr{p}_{li}_{ci}")
                           for p in range(nps)]

                    def pslot(m):
                        return psl[m // cpp], (m % cpp) * bs

                    # ident x_gates injections first: they have no h
                    # dependency, so PE executes them during the previous
                    # level's AllGather.  start=True marks the whole 2KB psum
                    # zero-region pending-zero, so only the FIRST matmul into
                    # each psum tile may set it; later ones overwrite their
                    # still-pending bytes.
                    for m in range(6):
                        pt, base = pslot(m)
                        first = m % cpp == 0
                        for (a, st, cnt, pos) in node_runs:
                            reg, loc = a // FTW, a % FTW
                            nc.tensor.matmul(
                                pt[:, base + pos:base + pos + cnt],
                                lhsT=ident_sb[:, :],
                                rhs=_c3(xgr[reg], m * FTW + loc, cnt, st),
                                start=first, stop=False,
                                skip_group_check=True)
                            first = False
                    for runs in (ch["l_src"], ch["r_src"]):
                        for (sli, sci, rel, st, cnt, pos) in runs:
                            load_tmp((sli, sci))
                    for m in range(6):
                        pt, base = pslot(m)
                        for c in range(KC):
                            runs = ch["l_src"] if c < HC else ch["r_src"]
                            hch = c if c < HC else c - HC
                            last = c == KC - 1
                            for (sli, sci, rel, st, cnt, pos) in runs:
                                ttile, tbs, _ago = tmps[(sli, sci)]
                                nc.tensor.matmul(
                                    pt[:, base + pos:base + pos + cnt],
                                    lhsT=wlr_v[:, c, m * HS:(m + 1) * HS],
                                    rhs=_c3(ttile, hch * (tbs + 1) + rel,
                                            cnt, st),
                                    start=False, stop=last,
                                    skip_group_check=True)

                    # activations: contiguous same-func spans per psum tile
                    for p in range(nps):
                        m0 = p * cpp
                        m1 = min(6, m0 + cpp)
                        msig = [m for m in range(m0, m1) if m != G_U]
                        mtan = [m for m in range(m0, m1) if m == G_U]
                        for ms, fn in ((msig, AF.Sigmoid), (mtan, AF.Tanh)):
                            if not ms:
                                continue
                            lo, hi = ms[0], ms[-1]
                            nc.scalar.activation(
                                g[:, lo * bs:(hi + 1) * bs],
                                psl[p][:, (lo - m0) * bs:(hi + 1 - m0) * bs],
                                fn)

                def gs(m):
                    return g[:, m * bs:(m + 1) * bs]

                at = ep.tile([HS, gw], F32, tag="ta", name=f"ta_{li}_{ci}")
                bt = ep.tile([HS, gw], F32, tag="tb", name=f"tb_{li}_{ci}")
                st_ = sp.tile([HS, gw], BF16, tag="st", name=f"st_{li}_{ci}")

                # c = ig*u (+ fl*c_l + fr*c_r), written straight into cT
                if li == 0:
                    for (a, stp, cnt, pos) in node_runs:
                        nc.vector.tensor_mul(
                            _c3(cT, a, cnt, stp),
                            _c3(g, G_I * bs + pos, cnt),
                            _c3(g, G_U * bs + pos, cnt))
                else:
                    nc.vector.tensor_mul(at[:, :bs], gs(G_I), gs(G_U))
                    for (a, stp, cnt, pos) in ch["l_runs"]:
                        nc.vector.tensor_mul(
                            _c3(bt, pos, cnt), _c3(g, G_FL * bs + pos, cnt),
                            _c3(cT, a, cnt, stp))
                    nc.vector.tensor_add(at[:, :bs], at[:, :bs], bt[:, :bs])
                    for (a, stp, cnt, pos) in ch["r_runs"]:
                        nc.vector.tensor_mul(
                            _c3(bt, pos, cnt), _c3(g, G_FR * bs + pos, cnt),
                            _c3(cT, a, cnt, stp))
                    for (a, stp, cnt, pos) in node_runs:
                        nc.vector.tensor_add(
                            _c3(cT, a, cnt, stp), _c3(at, pos, cnt),
                            _c3(bt, pos, cnt))
                # h = og * tanh(c); hf = rr*(h - px) + px  (bf16, to stage)
                for (a, stp, cnt, pos) in node_runs:
                    nc.scalar.activation(
                        _c3(bt, pos, cnt), _c3(cT, a, cnt, stp), AF.Tanh)
                nc.vector.tensor_mul(at[:, :bs], gs(G_O), bt[:, :bs])
                for (a, stp, cnt, pos) in node_runs:
                    reg, loc = a // FTW, a % FTW
                    nc.vector.tensor_sub(
                        _c3(bt, pos, cnt), _c3(at, pos, cnt),
                        _c3(pxr[reg], loc, cnt, stp))
                nc.vector.tensor_mul(bt[:, :bs], bt[:, :bs], gs(G_R))
                for (a, stp, cnt, pos) in node_runs:
                    reg, loc = a // FTW, a % FTW
                    nc.vector.tensor_add(
                        _c3(st_, pos, cnt), _c3(bt, pos, cnt),
                        _c3(pxr[reg], loc, cnt, stp))
                    # final output columns for this chunk
                    nc.sync.dma_start(
                        out=_c3(out, a, cnt, stp), in_=_c3(st_, pos, cnt))

                if li == nlev - 1 and ci == len(levels[li]["chunks"]) - 1:
                    return  # nothing consumes the last chunk's h
                # AllGather this chunk's h slice into every core's copy of
                # the chunk h tile.  Stage DMA + trigger from gpsimd (single
                # engine wake); gathered-output DMA from scalar.
                agi = dp.tile([HS, bs], BF16, tag="agin", name=f"agi_{li}_{ci}")
                ago = dp.tile([H, bs], BF16, tag="agout", name=f"ago_{li}_{ci}",
                              addr_space="Shared")
                nc.gpsimd.dma_start(out=agi[:, :], in_=st_[:, :bs])
                nc.gpsimd.collective_compute(
                    "AllGather", ALU.bypass,
                    replica_groups=[list(range(NCORES))],
                    ins=[agi.opt()], outs=[ago.opt()])
                hw = hwA if li < 2 else hwB
                pool = hp if li < 2 else hq
                ttile = pool.tile([128, HC * hw + 16], BF16,
                                  tag="hA" if li < 2 else "hB",
                                  name=f"h_{li}_{ci}")
                tmps[(li, ci)] = (ttile, bs, ago)
                t_v = ttile[:, 0:HC * (bs + 1)].rearrange(
                    "p (c b) -> p c b", c=HC)
                nc.vector.memset(t_v[:, :, bs:bs + 1], 0.0)

            # ---- emission: GEMM subtiles interleaved with level chunks.
            # tile_wait_until floors give the scheduler a model of the real
            # timeline so per-engine queue order matches the intended
            # pipeline (it otherwise hoists e.g. collective-output DMAs
            # above earlier chunks' activations).
            plan = list(feat_tiles)
            pi = 0
            t_us = 12.0
            for li, L in enumerate(levels):
                for ci in range(len(L["chunks"])):
                    while pi < len(plan) and plan[pi][2] <= (li, ci):
                        c0, w, _ = plan[pi]
                        pi += 1
                        with nc.named_scope(f"G{c0:04d}_{w}"), \
                                tc.tile_wait_until(t_us / 1000.0):
                            emit_gemm_tile(c0, w)
                        t_us += w * 0.062
                    ch = L["chunks"][ci]
                    with nc.named_scope(f"L{li:02d}c{ci}"), \
                            tc.tile_wait_until(t_us / 1000.0):
                        emit_chunk(li, ci, ch)
                    t_us += 2.0 if li == 0 else ch["bs"] * 0.055 + 3.0
                    if li >= 2:
                        t_us += 7.0
            while pi < len(plan):
                c0, w, _ = plan[pi]
                pi += 1
                with nc.named_scope(f"G{c0:04d}_{w}"), \
                        tc.tile_wait_until(t_us / 1000.0):
                    emit_gemm_tile(c0, w)
                t_us += w * 0.062
    return nc


# -------------------------------------------------------------- host logic --

def _prep(inputs, n, f):
    feats = np.asarray(inputs["features"], np.float32)
    wx = np.asarray(inputs["w_ioffux"], np.float32)
    bx = np.asarray(inputs["b_ioffux"], np.float32)
    wl = np.asarray(inputs["w_ioffuh_l"], np.float32)
    bl = np.asarray(inputs["b_ioffuh_l"], np.float32)
    wr = np.asarray(inputs["w_ioffuh_r"], np.float32)
    br = np.asarray(inputs["b_ioffuh_r"], np.float32)
    wpx = np.asarray(inputs["w_px"], np.float32)
    bpx = np.asarray(inputs["b_px"], np.float32)

    featT = np.ascontiguousarray(feats.T).astype(BF16_NP)
    identm = np.eye(128, dtype=BF16_NP)
    b_all = bx + bl + br

    in_maps = []
    for k in range(NCORES):
        rows = np.concatenate(
            [np.arange(t * H + k * HS, t * H + (k + 1) * HS) for t in GATE_PERM])
        wlr_T = np.ascontiguousarray(
            np.concatenate([wl[rows], wr[rows]], axis=1).T).astype(BF16_NP)
        wxpx = np.concatenate([wx[rows], wpx[k * HS:(k + 1) * HS]], axis=0)
        wxpx_T = np.ascontiguousarray(wxpx.T).astype(BF16_NP)
        b7 = np.concatenate([b_all[rows], bpx[k * HS:(k + 1) * HS]])
        biasm = np.ascontiguousarray(
            b7.reshape(7, HS).T).astype(np.float32)  # [128, 7]
        in_maps.append({
            "featT": featT,
            "wlrT": wlr_T,
            "wxpxT": wxpx_T,
            "biasT": biasm,
            "ident": identm,
        })
    return in_maps


def _assemble(results, n):
    out = np.empty((n, H), np.float32)
    for k in range(NCORES):
        out[:, k * HS:(k + 1) * HS] = \
            np.asarray(results[k]["out"]).astype(np.float32).T
    return out


_CACHE = {}


def _get_nc(inputs):
    feats = np.asarray(inputs["features"])
    n, f = feats.shape
    lc = np.asarray(inputs["left_child"])
    rc = np.asarray(inputs["right_child"])
    key = (n, f, lc.tobytes(), rc.tobytes())
    if key not in _CACHE:
        levels = _schedule(lc, rc, n)
        ftiles = _feat_tiles(levels, n)
        nc = bacc.Bacc(trn_type="TRN2", target_bir_lowering=False,
                       debug=False, num_devices=NCORES)
        build(nc, levels, ftiles, n, f)
        nc.compile()
        _CACHE[key] = nc
    return _CACHE[key], n, f


def kernel(**inputs):
    nc, n, f = _get_nc(inputs)
    in_maps = _prep(inputs, n, f)
    res = run_bass_kernel_spmd(nc, in_maps, core_ids=list(range(NCORES)))
    return _assemble(res.results, n)
